# revision 1
# baseline (speedup 1.0000x reference)
"""GATv2 attention-pool kernel for 8 Trainium2 NeuronCores.

Algorithm
---------
Reference computes, per edge e with target node t(e):
    feats = q + k                                   [E, 64]
    logits[e,h] = sum_c feats[e,h*8+c] * A[c,h]     [E, 8]
    attn = segment_softmax(logits, targets)         [E, 8]
    out[n] = relu(segment_sum(q * attn))            [N, 64]

Because logits are O(10), exp() never overflows fp32, so the segment-max
shift is unnecessary and softmax folds into two segment-SUMS that share
one pass:
    denom[n,h]  = sum_{e->n} exp(logits[e,h])
    pooled[n,:] = sum_{e->n} q[e,:] * exp(logits[e,h])
    out[n]      = relu(pooled[n]) / denom[n]        (denom > 0 always)

Distribution: edges are partitioned by target node (host-side sort), 100000
nodes split into 8 contiguous shards of 12500 -> all segment reductions are
core-local, no collectives.  Each shard is cut into 196 windows of 64 nodes;
a window's edges are padded to T_w * 128 slots (T_w identical across cores so
one SPMD program serves all 8 cores).  Per 128-edge subtile the device builds
a one-hot selector S[e, n_rel] = (rel[e] == n_rel) and accumulates
    psum[64, 72] += S^T @ [q*ex | ex]
on the PE across the window's subtiles, then divides / relus once per node.

Host work is index metadata + data layout only (argsort of targets, gather
of q/k rows into the sorted slot order); all floating-point math runs on
device.
"""

import os
import sys

import numpy as np

N_NODES = 100000
N_EDGES = 1600000
H = 8
C = 8
HC = H * C
N_CORES = 8
NODES_PER_CORE = N_NODES // N_CORES
WIN_NODES = 64
SUB = 128


def _ensure_imports():
    try:
        import concourse.bass  # noqa: F401
    except ImportError:
        for p in ("/opt/trn_rl_repo", "/root/.axon_site/_ro/trn_rl_repo"):
            if os.path.isdir(p) and p not in sys.path:
                sys.path.insert(0, p)


TSUB = 8  # subtiles per window: every window holds <= TSUB*SUB edges


def preprocess(targets, n_nodes, n_cores, win_nodes):
    """Sort edges by target; bin-pack each core's nodes into windows.

    Every window holds at most `win_nodes` nodes AND at most TSUB*SUB edges
    (two-pointer big+small pairing keeps fragmentation ~3%), so the device
    program is fully uniform: n_win windows of exactly TSUB subtiles.

    Returns (perms [n_cores, n_slots] edge ids, rels [n_cores, n_slots] f32,
    node_order [n_cores, n_win*win_nodes] int64 output-row -> node id (or -1),
    n_win, n_slots).
    """
    nodes_per_core = n_nodes // n_cores
    order = np.argsort(targets, kind="stable")
    tsorted = targets[order]
    node_start = np.searchsorted(tsorted, np.arange(n_nodes + 1))
    deg = np.diff(node_start)

    cap_e = TSUB * SUB
    # pack per core with a two-pointer over degree-sorted nodes
    packs = []   # per core: list of windows, each a list of node ids
    for c in range(n_cores):
        nodes = np.arange(c * nodes_per_core, (c + 1) * nodes_per_core)
        by_deg = nodes[np.argsort(deg[nodes], kind="stable")]
        lo, hi = 0, len(by_deg) - 1
        wins = []
        while lo <= hi:
            cur, cnt = [], 0
            # take the biggest remaining, then fill with smallest
            while lo <= hi and len(cur) < win_nodes:
                d = int(deg[by_deg[hi]])
                if cnt + d > cap_e:
                    break
                cur.append(by_deg[hi])
                cnt += d
                hi -= 1
                while lo <= hi and len(cur) < win_nodes:
                    d = int(deg[by_deg[lo]])
                    if cnt + d > cap_e:
                        break
                    cur.append(by_deg[lo])
                    cnt += d
                    lo += 1
            wins.append(cur)
        packs.append(wins)

    n_win = max(len(w) for w in packs)
    n_slots = n_win * cap_e
    perms = np.zeros((n_cores, n_slots), dtype=np.int64)
    rels = np.full((n_cores, n_slots), -1.0, dtype=np.float32)
    node_order = np.full((n_cores, n_win * win_nodes), -1, dtype=np.int64)
    for c in range(n_cores):
        for w, cur in enumerate(packs[c]):
            sb = w * cap_e
            pos = 0
            for j, node in enumerate(cur):
                e0, e1 = node_start[node], node_start[node + 1]
                cnt = e1 - e0
                perms[c, sb + pos:sb + pos + cnt] = order[e0:e1]
                rels[c, sb + pos:sb + pos + cnt] = j
                pos += cnt
                node_order[c, w * win_nodes + j] = node
    return perms, rels, node_order, n_win, n_slots


def build_nc(n_win, n_slots, out_rows):
    """Build the single SPMD Bass program for one core's shard."""
    _ensure_imports()
    import concourse.bacc as bacc
    import concourse.mybir as mybir
    import concourse.tile as tile

    f32 = mybir.dt.float32

    # process windows in pairs: one set of wide tiles per group amortizes
    # DVE per-op overhead and doubles DMA transfer sizes
    cap_e = TSUB * SUB
    groups = []
    w = 0
    while w < n_win:
        pair = [(w, TSUB, w * cap_e)]
        w += 1
        if w < n_win:
            pair.append((w, TSUB, w * cap_e))
            w += 1
        groups.append(pair)
    Tgmax = max(sum(t for _, t, _ in g) for g in groups)

    i16 = mybir.dt.int16
    bf16 = mybir.dt.bfloat16
    nc = bacc.Bacc("TRN2", num_devices=N_CORES)
    qk = nc.declare_dram_parameter("qk", [n_slots, 2 * HC], f32, False)
    rel = nc.declare_dram_parameter("rel", [n_slots], f32, False)
    wrow = nc.declare_dram_parameter("wrow", [128, Tgmax * HC], f32, False)
    iota16 = nc.declare_dram_parameter(
        "iota16", [128, Tgmax * WIN_NODES], i16, False)
    out = nc.declare_dram_parameter("out", [out_rows, HC], f32, isOutput=True)

    AX = mybir.AxisListType
    OP = mybir.AluOpType
    AF = mybir.ActivationFunctionType
    MW = 2 * HC  # qk row width

    with tile.TileContext(nc) as tc:
        with (
            tc.tile_pool(name="const", bufs=1) as cpool,
            tc.tile_pool(name="qk", bufs=5) as qkpool,
            tc.tile_pool(name="mid", bufs=4) as midpool,
            tc.tile_pool(name="mm", bufs=4) as mmpool,
            tc.tile_pool(name="fin", bufs=6) as finpool,
            tc.tile_pool(name="psum", bufs=8, space="PSUM") as ppool,
        ):
            w_t = cpool.tile([128, Tgmax * HC], f32)
            nc.sync.dma_start(out=w_t[:], in_=wrow[:])
            io_t = cpool.tile([128, Tgmax * WIN_NODES], i16)
            nc.sync.dma_start(out=io_t[:], in_=iota16[:])

            # software-pipelined by one group: the S-path and logits of
            # group i+1 are emitted between group i's exp/wq and its
            # epilogue, so ACT's FIFO runs exp_i, rr_{i+1}, sup_{i+1},
            # relu_i and never makes DVE wait on a long COPY.
            st = {}

            def emit_load(pair):
                Tg = sum(t for _, t, _ in pair)
                fd = Tg * HC
                qk_t = qkpool.tile([128, Tg * MW], f32, tag="qk")
                r_t = qkpool.tile([128, Tg], f32, tag="r")
                off = 0
                for _, Tw, wbase in pair:
                    nsl = Tw * SUB
                    nc.sync.dma_start(
                        out=qk_t[:, off * MW:(off + Tw) * MW],
                        in_=qk[wbase:wbase + nsl, :].rearrange(
                            "(p t) c -> p (t c)", p=128),
                    )
                    nc.sync.dma_start(
                        out=r_t[:, off:off + Tw],
                        in_=rel[wbase:wbase + nsl].rearrange(
                            "(p t) -> p t", p=128),
                    )
                    off += Tw
                qk3 = qk_t[:].rearrange("p (t c) -> p t c", c=MW)
                f_t = midpool.tile([128, fd], f32, tag="f")
                nc.vector.tensor_add(
                    f_t[:], qk3[:, :, 0:HC], qk3[:, :, HC:MW])
                return {"pair": pair, "Tg": Tg, "fd": fd, "qk3": qk3,
                        "f": f_t, "r": r_t}

            def emit_spath(s):
                Tg = s["Tg"]
                rr_t = mmpool.tile([128, Tg, WIN_NODES], i16, tag="rr")
                nc.scalar.activation(
                    out=rr_t[:],
                    in_=s["r"][:, :, None].to_broadcast(
                        [128, Tg, WIN_NODES]),
                    func=AF.Copy,
                )
                sb_t = mmpool.tile([128, Tg, WIN_NODES], bf16, tag="Sb")
                nc.vector.tensor_tensor(
                    out=sb_t[:],
                    in0=rr_t[:],
                    in1=io_t[:, :Tg * WIN_NODES].rearrange(
                        "p (t n) -> p t n", n=WIN_NODES),
                    op=OP.is_equal,
                )
                s_t = mmpool.tile([128, Tg, WIN_NODES], f32, tag="S")
                nc.scalar.activation(out=s_t[:], in_=sb_t[:], func=AF.Copy)
                s["S"] = s_t

            def emit_logits(s):
                Tg, fd = s["Tg"], s["fd"]
                wf_t = midpool.tile([128, fd], f32, tag="wf")
                nc.vector.tensor_mul(wf_t[:], s["f"][:], w_t[:, :fd])
                lg_t = midpool.tile([128, Tg * H], f32, tag="lg")
                nc.vector.tensor_reduce(
                    out=lg_t[:],
                    in_=wf_t[:].rearrange(
                        "p (t h c) -> p (t h) c", h=H, c=C),
                    axis=AX.X,
                    op=OP.add,
                )
                s["lg"] = lg_t

            def emit_exp_wq_mm(s):
                Tg = s["Tg"]
                m_t = mmpool.tile([128, Tg, H * C + H], f32, tag="M")
                nc.scalar.activation(
                    out=m_t[:, :, HC:HC + H],
                    in_=s["lg"][:].rearrange("p (t h) -> p t h", h=H),
                    func=AF.Exp,
                )
                nc.vector.tensor_mul(
                    m_t[:, :, 0:HC].rearrange("p t (h c) -> p t h c", h=H),
                    s["qk3"][:, :, 0:HC].rearrange(
                        "p t (h c) -> p t h c", h=H),
                    m_t[:, :, HC:HC + H, None].to_broadcast(
                        [128, Tg, H, C]),
                )
                pair = s["pair"]
                p_t = ppool.tile([WIN_NODES, len(pair) * (HC + H)], f32)
                off = 0
                for wi, (_, Tw, _) in enumerate(pair):
                    pcols = slice(wi * (HC + H), wi * (HC + H) + HC + H)
                    for g in range(Tw):
                        nc.tensor.matmul(
                            p_t[:, pcols],
                            lhsT=s["S"][:, off + g, :],
                            rhs=m_t[:, off + g, :],
                            start=(g == 0),
                            stop=(g == Tw - 1),
                        )
                    off += Tw
                s["psum"] = p_t

            def emit_epilogue(s):
                pair = s["pair"]
                nw = len(pair)
                p3 = s["psum"][:].rearrange("p (w j) -> p w j", j=HC + H)
                rc_t = finpool.tile([WIN_NODES, nw, H], f32, tag="rc")
                nc.vector.reciprocal(rc_t[:], p3[:, :, HC:HC + H])
                d_t = finpool.tile([WIN_NODES, nw, HC], f32, tag="d")
                nc.vector.tensor_mul(
                    d_t[:].rearrange("p w (h c) -> p w h c", h=H),
                    p3[:, :, 0:HC].rearrange("p w (h c) -> p w h c", h=H),
                    rc_t[:, :, :, None].to_broadcast(
                        [WIN_NODES, nw, H, C]),
                )
                o_t = finpool.tile([WIN_NODES, nw, HC], f32, tag="o")
                nc.scalar.activation(o_t[:], d_t[:], func=AF.Relu)
                w0 = pair[0][0]
                nc.sync.dma_start(
                    out=out[w0 * WIN_NODES:(w0 + nw) * WIN_NODES, :]
                    .rearrange("(w p) c -> p w c", w=nw),
                    in_=o_t[:],
                )

            cur = emit_load(groups[0])
            emit_spath(cur)
            emit_logits(cur)
            for gi in range(len(groups)):
                nxt = emit_load(groups[gi + 1]) if gi + 1 < len(groups) \
                    else None
                emit_exp_wq_mm(cur)
                if nxt is not None:
                    emit_spath(nxt)
                    emit_logits(nxt)
                emit_epilogue(cur)
                cur = nxt

    nc.finalize()
    return nc


def _host_arrays(query, key, attn_kernel, targets):
    perms, rels, node_order, n_win, n_slots = preprocess(
        targets, N_NODES, N_CORES, WIN_NODES
    )
    Tgmax = 2 * TSUB
    wrow_1 = np.ascontiguousarray(attn_kernel.T).reshape(-1)  # [h*8+c] = A[c,h]
    wrow = np.tile(wrow_1, (128, Tgmax)).astype(np.float32)
    iota16 = np.tile(
        np.arange(WIN_NODES, dtype=np.int16), (128, Tgmax)
    )
    in_maps = []
    for c in range(N_CORES):
        qkc = np.empty((n_slots, 2 * HC), dtype=np.float32)
        qkc[:, :HC] = query[perms[c]]
        qkc[:, HC:] = key[perms[c]]
        in_maps.append({
            "qk": qkc,
            "rel": rels[c],
            "wrow": wrow,
            "iota16": iota16,
        })
    return in_maps, node_order, n_win, n_slots


TRACE = False          # set by test harness to capture an NTFF profile
TRACE_CORES = None
LAST_RESULTS = None    # BassKernelResults of the most recent run


def kernel(query, key, attn_kernel, targets):
    global LAST_RESULTS
    query = np.asarray(query, dtype=np.float32)
    key = np.asarray(key, dtype=np.float32)
    attn_kernel = np.asarray(attn_kernel, dtype=np.float32)
    targets = np.asarray(targets, dtype=np.int32)

    _ensure_imports()
    from concourse.bass_utils import run_bass_kernel_spmd

    in_maps, node_order, n_win, n_slots = _host_arrays(
        query, key, attn_kernel, targets)
    out_rows = n_win * WIN_NODES
    nc = build_nc(n_win, n_slots, out_rows)
    res = run_bass_kernel_spmd(
        nc, in_maps, list(range(N_CORES)),
        trace=TRACE, trace_cores=TRACE_CORES,
    )
    LAST_RESULTS = res
    out = np.zeros((N_NODES, HC), dtype=np.float32)
    for c in range(N_CORES):
        rows = node_order[c]
        valid = rows >= 0
        out[rows[valid]] = res.results[c]["out"][valid]

    deg = np.bincount(targets, minlength=N_NODES)
    out[deg == 0] = 0.0
    return out



# revision 7
# speedup vs baseline: 2.3157x; 2.3157x over previous
"""GATv2 attention-pool kernel for 8 Trainium2 NeuronCores.

v2 "diagonal scatter" design
----------------------------
Reference computes, per edge e with target node t(e):
    feats = q + k                                   [E, 64]
    logits[e,h] = sum_c feats[e,h*8+c] * A[c,h]     [E, 8]
    attn = segment_softmax(logits, targets)         [E, 8]
    out[n] = relu(segment_sum(q * attn))            [N, 64]

Logits are O(10) so exp() never overflows fp32; the segment-max shift is
unnecessary and softmax folds into two segment-SUMS:
    denom[n,h]  = sum_{e->n} exp(logits[e,h])
    pooled[n,:] = sum_{e->n} q[e,:] * exp(logits[e,h])
    out[n]      = relu(pooled[n]) / denom[n]

Layout: nodes are sorted by degree and dealt round-robin to the 8 cores so
every core sees an identical degree profile (one SPMD program).  A window is
128 similar-degree nodes, ONE NODE PER SBUF PARTITION; each node's edge list
is padded to the window max Tw (~5% padding, pad rows are crafted so their
logits are ~-30 for every head -> exp ~ 1e-13 contributes nothing).  With
that layout the segment reductions are plain free-dim sums per partition:
    psum[128, 72] += I128^T @ m[:, t, 0:72]     (t = 0..Tw-1)
i.e. PSUM accumulation through the PE with a CONSTANT identity stationary
matrix - no per-subtile one-hot selector, no index tensors on device at all.
Everything ships and computes in bf16 (fp32 only for logits tail + PSUM).

Host work is index metadata + data layout only (degree sort, gather of q/k
rows into the padded slot order, bf16 cast); all floating-point math runs on
device.
"""

import os
import sys

import numpy as np

N_NODES = 100000
N_EDGES = 1600000
H = 8
C = 8
HC = H * C
N_CORES = 8
P = 128                       # nodes per window == SBUF partitions
NPC = N_NODES // N_CORES      # nodes per core
N_WIN = (NPC + P - 1) // P    # windows per core
TCAP = 80                     # max t-columns per processing group
NWCAP = 7                     # psum bank holds nw*72 fp32 <= 512
PAD_LG = -30.0                # logit forced onto pad slots


def _ensure_imports():
    try:
        import concourse.bass  # noqa: F401
    except ImportError:
        for p in ("/opt/trn_rl_repo", "/root/.axon_site/_ro/trn_rl_repo"):
            if os.path.isdir(p) and p not in sys.path:
                sys.path.insert(0, p)


def _run_arange(counts):
    """[0..c0-1, 0..c1-1, ...] for run lengths `counts`."""
    tot = int(counts.sum())
    a = np.arange(tot, dtype=np.int64)
    off = np.repeat(np.cumsum(counts) - counts, counts)
    return a - off


def preprocess(targets):
    """Degree-sort nodes, deal round-robin to cores, pack windows.

    Returns (order, deg, Tw [N_WIN], c0 [N_WIN+1], groups, C_total,
    idx_maps: per-core [P, C_total] int64 edge id or -1 for pad).
    """
    deg = np.bincount(targets, minlength=N_NODES).astype(np.int64)
    order = np.argsort(deg, kind="stable")          # ascending degree
    Tw = np.zeros(N_WIN, dtype=np.int64)
    for w in range(N_WIN):
        lo = w * P * N_CORES
        hi = min((w + 1) * P * N_CORES, N_NODES)
        Tw[w] = max(1, int(deg[order[lo:hi]].max()))
    c0 = np.zeros(N_WIN + 1, dtype=np.int64)
    c0[1:] = np.cumsum(Tw)
    C_total = int(c0[-1])

    groups = []  # (w0, nw, T, cstart)
    w = 0
    while w < N_WIN:
        w0 = w
        T = 0
        while w < N_WIN and (w - w0) < NWCAP and T + Tw[w] <= TCAP:
            T += int(Tw[w])
            w += 1
        if w == w0:          # single window wider than TCAP
            T = int(Tw[w])
            w += 1
        groups.append((w0, w - w0, T, int(c0[w0])))

    eorder = np.argsort(targets, kind="stable")
    tsorted = targets[eorder]
    estart = np.searchsorted(tsorted, np.arange(N_NODES + 1))

    idx_maps = []
    for cidx in range(N_CORES):
        nodes = order[cidx::N_CORES]                # local rank l -> node
        l = np.arange(len(nodes), dtype=np.int64)
        wloc = l // P
        ploc = l % P
        d = deg[nodes]
        dstart = ploc * C_total + c0[wloc]
        dst = np.repeat(dstart, d) + _run_arange(d)
        src = np.repeat(estart[nodes], d) + _run_arange(d)
        idx = np.full(P * C_total, -1, dtype=np.int64)
        idx[dst] = eorder[src]
        idx_maps.append(idx.reshape(P, C_total))
    return order, deg, Tw, c0, groups, C_total, idx_maps


def build_nc(groups, Tw, C_total):
    """Single SPMD Bass program for one core's shard."""
    _ensure_imports()
    import concourse.bacc as bacc
    import concourse.mybir as mybir
    import concourse.tile as tile

    f32 = mybir.dt.float32
    bf16 = mybir.dt.bfloat16
    f16 = mybir.dt.float16
    AF = mybir.ActivationFunctionType

    # fp16 for the small-range logits path (8x finer mantissa than bf16);
    # bf16 only where exp() range demands it (eb, m); fp32 logits + PSUM.
    nc = bacc.Bacc("TRN2", num_devices=N_CORES)
    qd = nc.declare_dram_parameter("q", [P, C_total * HC], f16, False)
    kd = nc.declare_dram_parameter("k", [P, C_total * HC], f16, False)
    wd = nc.declare_dram_parameter("w", [P, TCAP * HC], f16, False)
    idd = nc.declare_dram_parameter("ident", [P, P], bf16, False)
    outd = nc.declare_dram_parameter("out", [P, N_WIN * HC], f16,
                                     isOutput=True)

    with tile.TileContext(nc) as tc:
        with (
            tc.tile_pool(name="const", bufs=1) as cpool,
            tc.tile_pool(name="qin", bufs=3) as qpool,
            tc.tile_pool(name="kin", bufs=3) as kpool,
            tc.tile_pool(name="mid", bufs=2) as midpool,
            tc.tile_pool(name="mm", bufs=2) as mpool,
            tc.tile_pool(name="fin", bufs=3) as finpool,
            tc.tile_pool(name="psum", bufs=8, space="PSUM") as ppool,
        ):
            w_t = cpool.tile([P, TCAP * HC], f16)
            nc.sync.dma_start(out=w_t[:], in_=wd[:])
            id_t = cpool.tile([P, P], bf16)
            nc.sync.dma_start(out=id_t[:], in_=idd[:])

            for (w0, nw, T, cs) in groups:
                fd = T * HC
                qt = qpool.tile([P, fd], f16, tag="q")
                nc.sync.dma_start(out=qt[:], in_=qd[:, cs * HC:(cs + T) * HC])
                kt = kpool.tile([P, fd], f16, tag="k")
                nc.sync.dma_start(out=kt[:], in_=kd[:, cs * HC:(cs + T) * HC])

                ft = midpool.tile([P, fd], f16, tag="f")
                nc.vector.tensor_add(ft[:], qt[:], kt[:])
                wf = midpool.tile([P, fd], f16, tag="wf")
                nc.vector.tensor_mul(wf[:], ft[:], w_t[:, :fd])

                # tree-reduce the 8 channels per (t, head)
                wf4 = wf[:].rearrange("p (x c) -> p x c", c=8)
                t1 = midpool.tile([P, T * 8, 4], f16, tag="t1")
                nc.vector.tensor_add(t1[:], wf4[:, :, 0:4], wf4[:, :, 4:8])
                t2 = midpool.tile([P, T * 8, 2], f16, tag="t2")
                nc.vector.tensor_add(t2[:], t1[:, :, 0:2], t1[:, :, 2:4])
                lg = midpool.tile([P, T * 8], f32, tag="lg")
                nc.vector.tensor_add(lg[:], t2[:, :, 0], t2[:, :, 1])

                lg3 = lg[:].rearrange("p (t h) -> p t h", h=H)
                eb = midpool.tile([P, fd], bf16, tag="eb")
                nc.scalar.activation(
                    out=eb[:].rearrange("p (t h c) -> p t h c", h=H, c=C),
                    in_=lg3[:, :, :, None].to_broadcast([P, T, H, C]),
                    func=AF.Exp,
                )

                m = mpool.tile([P, T, 72], bf16, tag="m")
                nc.vector.tensor_mul(
                    m[:, :, 0:HC],
                    qt[:].rearrange("p (t j) -> p t j", j=HC),
                    eb[:].rearrange("p (t j) -> p t j", j=HC))
                nc.scalar.activation(out=m[:, :, HC:72], in_=lg3, func=AF.Exp)

                ps = ppool.tile([P, nw * 72], f32)
                tg = 0
                for wi in range(nw):
                    tw = int(Tw[w0 + wi])
                    for t in range(tw):
                        nc.tensor.matmul(
                            ps[:, wi * 72:(wi + 1) * 72],
                            lhsT=id_t[:],
                            rhs=m[:, tg + t, :],
                            start=(t == 0),
                            stop=(t == tw - 1),
                        )
                    tg += tw

                ps3 = ps[:].rearrange("p (w j) -> p w j", j=72)
                o = finpool.tile([P, nw, HC], f32, tag="o")
                nc.scalar.activation(out=o[:], in_=ps3[:, :, 0:HC],
                                     func=AF.Relu)
                rc = finpool.tile([P, nw, H], f32, tag="rc")
                nc.vector.reciprocal(rc[:], ps3[:, :, HC:72])
                o2 = finpool.tile([P, nw, HC], f16, tag="o2")
                nc.vector.tensor_mul(
                    o2[:].rearrange("p w (h c) -> p w h c", h=H),
                    o[:].rearrange("p w (h c) -> p w h c", h=H),
                    rc[:, :, :, None].to_broadcast([P, nw, H, C]),
                )
                nc.sync.dma_start(
                    out=outd[:, w0 * HC:(w0 + nw) * HC],
                    in_=o2[:].rearrange("p w j -> p (w j)"),
                )

    nc.finalize()
    return nc


def _host_arrays(query, key, attn_kernel, targets):
    import ml_dtypes

    bf = ml_dtypes.bfloat16
    f16 = np.float16
    order, deg, Tw, c0, groups, C_total, idx_maps = preprocess(targets)

    qb = query.astype(f16)
    kb = key.astype(f16)

    # pad rows: q = 0; k chosen so logits[h] == PAD_LG for every head
    A = attn_kernel.astype(np.float64)               # [C, H]
    nrm = np.maximum((A * A).sum(axis=0), 1e-6)      # ||A[:,h]||^2
    v = (PAD_LG / nrm)[None, :] * A                  # [C, H]
    kpad_row = np.ascontiguousarray(v.T).reshape(-1).astype(f16)  # [h*8+c]

    wrow = np.ascontiguousarray(attn_kernel.T).reshape(-1)  # [h*8+c] = A[c,h]
    w_arr = np.tile(wrow, (P, TCAP)).astype(f16)
    ident = np.eye(P, dtype=np.float32).astype(bf)

    in_maps = []
    for cidx in range(N_CORES):
        idx = idx_maps[cidx]
        safe = np.maximum(idx, 0)
        pad = idx < 0
        qdev = qb[safe]
        qdev[pad] = 0
        kdev = kb[safe]
        kdev[pad] = kpad_row
        in_maps.append({
            "q": np.ascontiguousarray(qdev.reshape(P, C_total * HC)),
            "k": np.ascontiguousarray(kdev.reshape(P, C_total * HC)),
            "w": w_arr,
            "ident": ident,
        })
    return in_maps, order, deg, Tw, groups, C_total


TRACE = False          # set by test harness to capture an NTFF profile
TRACE_CORES = None
LAST_RESULTS = None    # BassKernelResults of the most recent run


def kernel(query, key, attn_kernel, targets):
    global LAST_RESULTS
    query = np.asarray(query, dtype=np.float32)
    key = np.asarray(key, dtype=np.float32)
    attn_kernel = np.asarray(attn_kernel, dtype=np.float32)
    targets = np.asarray(targets, dtype=np.int32)

    _ensure_imports()
    from concourse.bass_utils import run_bass_kernel_spmd

    in_maps, order, deg, Tw, groups, C_total = _host_arrays(
        query, key, attn_kernel, targets)
    nc = build_nc(groups, Tw, C_total)
    res = run_bass_kernel_spmd(
        nc, in_maps, list(range(N_CORES)),
        trace=TRACE, trace_cores=TRACE_CORES,
    )
    LAST_RESULTS = res

    out = np.zeros((N_NODES, HC), dtype=np.float32)
    for cidx in range(N_CORES):
        r = np.asarray(res.results[cidx]["out"], dtype=np.float32)
        r = r.reshape(P, N_WIN, HC)
        nodes = order[cidx::N_CORES]
        l = np.arange(len(nodes), dtype=np.int64)
        out[nodes] = r[l % P, l // P, :]
    out[deg == 0] = 0.0
    return out
